# revision 39
# baseline (speedup 1.0000x reference)
"""LinearAttention Trainium2 kernel (8 NeuronCores, sequence-sharded).

Reference computation (per batch b):
    qkv = x @ W_qkv; q,k,v split; per-head: softmax(q, dim=dh),
    softmax(k, dim=seq); ctx = k^T v; out = q_sm @ ctx; y = out @ W_out + b.

v4 dataflow per core (sequence shard of 1024 rows x 2 batches):
  phase 1 (per 128-row tile, software-pipelined one deep):
      qkv = xT.T @ W (24 N=512 matmuls per tile, f32 PSUM);
      exp_k / v-copy / exp_q on scalar; per-head q sums via one segmented
      DVE reduce + reciprocal; per-head scale split scalar/vector;
      previous tile's ctxT/Z matmuls + q_sm PE transposes issue under the
      current tile's matmul chain so the PE never waits.
      AllReduce of [ctxT | Z] issued right after each batch's tiles.
  phase 2 (per batch): MZ_h = (ctx_h @ W_out_h)/Z with 1/Z folded into
      the PSUM->SBUF copy scale; y = sum_t qsmT_t^T @ MZ_t.
Host: shards/transposes/casts x, gathers per-core y shards, adds b_out.
"""
import numpy as np
import ml_dtypes
from contextlib import ExitStack

import concourse.bass as bass
import concourse.mybir as mybir
import concourse.tile as tile
from concourse import bacc
from concourse.bass_utils import run_bass_kernel_spmd
from concourse.masks import make_identity

bf16 = ml_dtypes.bfloat16
F32 = mybir.dt.float32
BF = mybir.dt.bfloat16
EXP = mybir.ActivationFunctionType.Exp
COPY = mybir.ActivationFunctionType.Copy
ADD = mybir.AluOpType.add
AX_X = mybir.AxisListType.X

B, N, D = 2, 8192, 1024
H, DH, INNER = 8, 64, 512
NCORES = 8
NL = N // NCORES            # 1024 seq rows per batch per core
SEQ = B * NL                # 2048 rows per core
NT_B = NL // 128            # 8 seq (128-row) tiles per batch


def _body(tc, xT, wq, wo, y):
    nc = tc.nc
    with ExitStack() as ctx:
        const = ctx.enter_context(tc.tile_pool(name="const", bufs=1))
        dram = ctx.enter_context(tc.tile_pool(name="dram", bufs=1, space="DRAM"))
        work = ctx.enter_context(tc.tile_pool(name="work", bufs=2))
        small = ctx.enter_context(tc.tile_pool(name="small", bufs=2))

        ident = const.tile([128, 128], BF)
        make_identity(nc, ident)
        ones_bf = const.tile([128, 1], BF)
        nc.vector.memset(ones_bf, 1.0)

        # interleave weight and xT loads so the first tiles start early
        xt = const.tile([128, 8, SEQ], BF)           # resident xT
        wq_sb = const.tile([128, 8, 3 * INNER], BF)
        wo_sb = const.tile([128, 4, D], BF)
        xT_r = xT[:].rearrange("(c p) s -> p c s", p=128)
        for kk in range(8):
            nc.sync.dma_start(out=wq_sb[:, kk, :],
                              in_=wq[128 * kk:128 * (kk + 1), :])
            nc.sync.dma_start(out=xt[:, kk, 0:256], in_=xT_r[:, kk, 0:256])
        for kk in range(8):
            nc.sync.dma_start(out=xt[:, kk, 256:NL], in_=xT_r[:, kk, 256:NL])
        for kk in range(8):
            nc.sync.dma_start(out=xt[:, kk, NL:SEQ], in_=xT_r[:, kk, NL:SEQ])
        for t in range(4):
            nc.sync.dma_start(out=wo_sb[:, t, :], in_=wo[128 * t:128 * (t + 1), :])

        qsmT = const.tile([128, 4, SEQ], BF)   # persistent q_sm^T
        cz_acc = []
        for b in range(B):
            cz_b = const.tile([128, 260], F32, tag=f"cz{b}", name=f"cz_acc{b}")
            nc.vector.memset(cz_b, 0.0)
            cz_acc.append(cz_b)

        red = []      # allreduced [ctxT | Z] per batch
        red_sb = []   # SBUF copies
        # qsm tiles outlive phase 1 (all transposes are deferred into the
        # AllReduce wait window), so pool at top level.
        qsm_pool = ctx.enter_context(tc.tile_pool(name="qsm", bufs=19))
        deferred_tr = {0: [], 1: []}   # (st, qsm) per batch

        def tile_transpose(b, st, qsm, pool):
            s0 = b * NL + st * 128
            trp = pool.tile([128, 4, 128], BF, tag="tr", name="trp")
            for c in range(4):
                nc.tensor.transpose(trp[:, c, :], qsm[:, 128 * c:128 * (c + 1)],
                                    ident)
            for c in range(4):
                nc.vector.tensor_copy(out=qsmT[:, c, s0:s0 + 128],
                                      in_=trp[:, c, :])

        # ---- phase 1: qkv + softmaxes + ctx/Z, tiles pipelined one deep ----
        with ExitStack() as p12:
            kvq_ps = p12.enter_context(tc.tile_pool(name="kvq_ps", bufs=2, space="PSUM"))
            cz_ps = p12.enter_context(tc.tile_pool(name="cz_ps", bufs=1, space="PSUM"))
            ek_pool = p12.enter_context(tc.tile_pool(name="ek", bufs=3))
            v_pool = p12.enter_context(tc.tile_pool(name="vp", bufs=3))
            eq_pool = p12.enter_context(tc.tile_pool(name="eq", bufs=3))
            qs_pool = p12.enter_context(tc.tile_pool(name="qs", bufs=3))

            def tile_mms(b, st):
                s0 = b * NL + st * 128
                kvq = kvq_ps.tile([128, 1536], F32, tag="kvq", name="kvq")
                for kk in range(8):
                    first, last = (kk == 0), (kk == 7)
                    nc.tensor.matmul(kvq[:, 0:512], lhsT=xt[:, kk, s0:s0 + 128],
                                     rhs=wq_sb[:, kk, 512:1024],
                                     start=first, stop=last)
                    nc.tensor.matmul(kvq[:, 512:1024], lhsT=xt[:, kk, s0:s0 + 128],
                                     rhs=wq_sb[:, kk, 1024:1536],
                                     start=first, stop=last)
                    nc.tensor.matmul(kvq[:, 1024:1536], lhsT=xt[:, kk, s0:s0 + 128],
                                     rhs=wq_sb[:, kk, 0:512],
                                     start=first, stop=last)
                return kvq

            def tile_elem(b, st, kvq):
                expk = ek_pool.tile([128, INNER], BF, tag="expk", name="expk")
                nc.scalar.activation(out=expk, in_=kvq[:, 0:512], func=EXP)
                vsb = v_pool.tile([128, INNER], BF, tag="v", name="vsb")
                nc.scalar.copy(out=vsb, in_=kvq[:, 512:1024])
                expq = eq_pool.tile([128, 8, 64], BF, tag="eq", name="expq")
                nc.scalar.activation(out=expq, in_=kvq[:, 1024:1536], func=EXP)
                qsum = qs_pool.tile([128, 8], F32, tag="qsum", name="qsum")
                nc.vector.tensor_reduce(qsum, expq, axis=AX_X, op=ADD)
                rq = qs_pool.tile([128, 8], F32, tag="rq", name="rq")
                nc.vector.reciprocal(rq, qsum)
                qsm = qsm_pool.tile([128, INNER], BF, tag="qsm", name="qsm")
                for h in range(H):
                    if h % 2 == 0:
                        nc.vector.tensor_scalar_mul(
                            qsm[:, 64 * h:64 * (h + 1)], expq[:, h, :],
                            rq[:, h:h + 1])
                    else:
                        nc.scalar.activation(
                            out=qsm[:, 64 * h:64 * (h + 1)], in_=expq[:, h, :],
                            func=COPY, scale=rq[:, h:h + 1])
                return expk, vsb, qsm

            def tile_deferred(b, st, expk, vsb, qsm):
                # ctx/Z matmuls for a finished tile; transposes inline for
                # batch 0, deferred into the AR-b1 window for batch 1
                cz = cz_ps.tile([128, 260], F32, tag="cz", name="cz")
                for h in range(H):
                    t, r = h // 2, h % 2
                    nc.tensor.matmul(
                        cz[64 * r:64 * (r + 1), 64 * t:64 * (t + 1)],
                        lhsT=vsb[:, 64 * h:64 * (h + 1)],
                        rhs=expk[:, 64 * h:64 * (h + 1)],
                        start=True, stop=True)
                for j in range(4):
                    nc.tensor.matmul(
                        cz[:, 256 + j:257 + j],
                        lhsT=expk[:, 128 * j:128 * (j + 1)], rhs=ones_bf,
                        start=True, stop=True)
                nc.vector.tensor_add(cz_acc[b], cz_acc[b], cz)
                deferred_tr[b].append((st, qsm))

            prev = None
            for b in range(B):
                for st in range(NT_B):
                    kvq = tile_mms(b, st)
                    if prev is not None:
                        tile_deferred(*prev)
                    ev = tile_elem(b, st, kvq)
                    prev = (b, st, *ev)
                tile_deferred(*prev)   # flush so the AR can be issued
                prev = None

                # keep collective-gated DMAs off the sync queue so output
                # DMAs emitted later never wait behind the AllReduce
                czbf = work.tile([128, 260], BF, tag=f"czbf{b}", name=f"czbf{b}")
                nc.vector.tensor_copy(out=czbf, in_=cz_acc[b])
                part_b = dram.tile([128, 260], BF, tag=f"part{b}", name=f"part{b}")
                red_b = dram.tile([128, 260], BF, tag=f"red{b}", name=f"red{b}")
                nc.gpsimd.dma_start(out=part_b, in_=czbf)
                nc.gpsimd.collective_compute(
                    "AllReduce", mybir.AluOpType.add,
                    replica_groups=[list(range(NCORES))],
                    ins=[part_b.opt()], outs=[red_b.opt()])
                red.append(red_b)
                red_c = work.tile([128, 260], BF, tag=f"red{b}", name=f"red_sb{b}")
                nc.gpsimd.dma_start(out=red_c, in_=red_b)
                red_sb.append(red_c)

        # ---- phase 2: M = (ctx @ W_out)/Z ; y = qsmT^T @ MZ ----
        with ExitStack() as pout:
            m_ps = pout.enter_context(tc.tile_pool(name="m_ps", bufs=2, space="PSUM"))
            y_ps = pout.enter_context(tc.tile_pool(name="y_ps", bufs=3, space="PSUM"))
            tr2_ps = pout.enter_context(tc.tile_pool(name="tr2_ps", bufs=2, space="PSUM"))
            ysb_pool = pout.enter_context(tc.tile_pool(name="ysb", bufs=4))

            def m_phase(b):
                rz = small.tile([128, 4], F32, tag="rz", name="rz")
                nc.vector.reciprocal(rz, red_sb[b][:, 256:260])
                m_sb = work.tile([128, 4, D], BF, tag="msb", name="m_sb")
                for t in range(4):
                    for cb in range(2):
                        mp = m_ps.tile([128, 512], F32, tag="mp", name="mp")
                        for r in range(2):
                            nc.tensor.matmul(
                                mp[64 * r:64 * (r + 1), :],
                                lhsT=red_sb[b][64 * r:64 * (r + 1), 64 * t:64 * (t + 1)],
                                rhs=wo_sb[64 * r:64 * (r + 1), t, cb * 512:(cb + 1) * 512],
                                start=True, stop=True)
                        if cb == 0:
                            nc.vector.tensor_scalar_mul(
                                m_sb[:, t, 0:512], mp, rz[:, t:t + 1])
                        else:
                            nc.scalar.activation(
                                out=m_sb[:, t, 512:1024], in_=mp,
                                func=COPY, scale=rz[:, t:t + 1])
                return m_sb

            def y_phase(b, m_sb):
                for mi in range(NT_B):
                    ysb = ysb_pool.tile([128, D], F32, tag="ysb", name="ysb")
                    for cb in range(2):
                        yp = y_ps.tile([128, 512], F32, tag="yp", name="yp")
                        for t in range(4):
                            nc.tensor.matmul(
                                yp, lhsT=qsmT[:, t, b * NL + mi * 128:
                                              b * NL + (mi + 1) * 128],
                                rhs=m_sb[:, t, cb * 512:(cb + 1) * 512],
                                start=(t == 0), stop=(t == 3))
                        if cb == 0:
                            nc.vector.tensor_copy(
                                out=ysb[:, 0:512], in_=yp)
                        else:
                            nc.scalar.copy(
                                out=ysb[:, 512:1024], in_=yp)
                        nc.sync.dma_start(
                            out=y[b * NL + mi * 128: b * NL + (mi + 1) * 128,
                                  cb * 512:(cb + 1) * 512],
                            in_=ysb[:, cb * 512:(cb + 1) * 512])

            # AR-b1 fill: M0 + all 16 tiles' transposes + y0 (~34us of PE)
            # cover the AR latency so M1/y1 start warm with no stall
            m0 = m_phase(0)
            for b in range(B):
                for st, qsm in deferred_tr[b]:
                    tile_transpose(b, st, qsm, tr2_ps)
            y_phase(0, m0)
            m1 = m_phase(1)
            y_phase(1, m1)


_COMPILED = None


def _build():
    global _COMPILED
    if _COMPILED is None:
        nc = bacc.Bacc("TRN2", target_bir_lowering=False, debug=False,
                       num_devices=NCORES)
        xT = nc.declare_dram_parameter("xT", [D, SEQ], BF, isOutput=False)
        wq = nc.declare_dram_parameter("wq", [D, 3 * INNER], BF, isOutput=False)
        wo = nc.declare_dram_parameter("wo", [INNER, D], BF, isOutput=False)
        y = nc.declare_dram_parameter("y", [SEQ, D], F32, isOutput=True)
        with tile.TileContext(nc) as tc:
            _body(tc, xT, wq, wo, y)
        nc.compile()
        _COMPILED = nc
    return _COMPILED


def _make_in_maps(x, W_qkv, W_out):
    wq_bf = np.ascontiguousarray(W_qkv).astype(bf16)
    wo_bf = np.ascontiguousarray(W_out).astype(bf16)
    in_maps = []
    for c in range(NCORES):
        rows = slice(c * NL, (c + 1) * NL)
        xs = np.concatenate([x[0, rows], x[1, rows]], axis=0)  # [2048, 1024]
        xT_bf = np.ascontiguousarray(xs.T).astype(bf16)        # [1024, 2048]
        in_maps.append({"xT": xT_bf, "wq": wq_bf, "wo": wo_bf})
    return in_maps


def _run(x, W_qkv, W_out, b_out, trace=False, **spmd_kwargs):
    nc = _build()
    in_maps = _make_in_maps(x, W_qkv, W_out)
    res = run_bass_kernel_spmd(nc, in_maps, list(range(NCORES)),
                               trace=trace, **spmd_kwargs)
    out = np.empty((B, N, D), np.float32)
    for c in range(NCORES):
        yc = res.results[c]["y"]
        rows = slice(c * NL, (c + 1) * NL)
        out[0, rows] = yc[:NL]
        out[1, rows] = yc[NL:]
    out += np.asarray(b_out, np.float32)[None, None, :]
    return out, res


def kernel(x, W_qkv, W_out, b_out):
    x = np.asarray(x, np.float32)
    out, _ = _run(x, np.asarray(W_qkv, np.float32),
                  np.asarray(W_out, np.float32),
                  np.asarray(b_out, np.float32))
    return out


# revision 40
# speedup vs baseline: 1.0424x; 1.0424x over previous
"""LinearAttention Trainium2 kernel (8 NeuronCores, sequence-sharded).

Reference computation (per batch b):
    qkv = x @ W_qkv; q,k,v split; per-head: softmax(q, dim=dh),
    softmax(k, dim=seq); ctx = k^T v; out = q_sm @ ctx; y = out @ W_out + b.

v4 dataflow per core (sequence shard of 1024 rows x 2 batches):
  phase 1 (per 128-row tile, software-pipelined one deep):
      qkv = xT.T @ W (24 N=512 matmuls per tile off one resident-xT
      LDWEIGHTS per k-chunk, f32 PSUM); exp_k / v-copy / exp_q on scalar;
      per-head q sums via one segmented DVE reduce + reciprocal; per-head
      scale split scalar/vector; the previous tile's ctxT/Z matmuls issue
      under the current tile's matmul chain so the PE never waits.
      A bf16 AllReduce of [ctxT | Z] is issued right after each batch's
      tiles (collective-gated DMAs ride the gpsimd queue so output DMAs
      never queue behind the collective).
  phase 2: M0 matmuls, then ALL 16 tiles' q_sm PE transposes (deferred
      from phase 1), then y0 — ~32us of PE work that covers the AR-b1
      latency so M1/y1 start warm with no stall. MZ_h = (ctx_h@W_out_h)/Z
      with 1/Z folded into the PSUM->SBUF copy scale; y = qsmT_t^T @ MZ_t
      summed over t; y copies alternate vector/scalar engines.
Host: shards/transposes/casts x, gathers per-core y shards, adds b_out.
"""
import numpy as np
import ml_dtypes
from contextlib import ExitStack

import concourse.bass as bass
import concourse.mybir as mybir
import concourse.tile as tile
from concourse import bacc
from concourse.bass_utils import run_bass_kernel_spmd
from concourse.masks import make_identity

bf16 = ml_dtypes.bfloat16
F32 = mybir.dt.float32
BF = mybir.dt.bfloat16
EXP = mybir.ActivationFunctionType.Exp
COPY = mybir.ActivationFunctionType.Copy
ADD = mybir.AluOpType.add
AX_X = mybir.AxisListType.X

B, N, D = 2, 8192, 1024
H, DH, INNER = 8, 64, 512
NCORES = 8
NL = N // NCORES            # 1024 seq rows per batch per core
SEQ = B * NL                # 2048 rows per core
NT_B = NL // 128            # 8 seq (128-row) tiles per batch


def _body(tc, xT, wq, wo, y):
    nc = tc.nc
    with ExitStack() as ctx:
        const = ctx.enter_context(tc.tile_pool(name="const", bufs=1))
        dram = ctx.enter_context(tc.tile_pool(name="dram", bufs=1, space="DRAM"))
        work = ctx.enter_context(tc.tile_pool(name="work", bufs=2))
        small = ctx.enter_context(tc.tile_pool(name="small", bufs=2))

        ident = const.tile([128, 128], BF)
        make_identity(nc, ident)
        ones_bf = const.tile([128, 1], BF)
        nc.vector.memset(ones_bf, 1.0)

        # interleave weight and xT loads so the first tiles start early
        xt = const.tile([128, 8, SEQ], BF)           # resident xT
        wq_sb = const.tile([128, 8, 3 * INNER], BF)
        wo_sb = const.tile([128, 4, D], BF)
        xT_r = xT[:].rearrange("(c p) s -> p c s", p=128)
        for kk in range(8):
            nc.sync.dma_start(out=wq_sb[:, kk, :],
                              in_=wq[128 * kk:128 * (kk + 1), :])
            nc.sync.dma_start(out=xt[:, kk, 0:256], in_=xT_r[:, kk, 0:256])
        for kk in range(8):
            nc.sync.dma_start(out=xt[:, kk, 256:NL], in_=xT_r[:, kk, 256:NL])
        for kk in range(8):
            nc.sync.dma_start(out=xt[:, kk, NL:SEQ], in_=xT_r[:, kk, NL:SEQ])
        for t in range(4):
            nc.sync.dma_start(out=wo_sb[:, t, :], in_=wo[128 * t:128 * (t + 1), :])

        qsmT = const.tile([128, 4, SEQ], BF)   # persistent q_sm^T
        cz_acc = []
        for b in range(B):
            cz_b = const.tile([128, 260], F32, tag=f"cz{b}", name=f"cz_acc{b}")
            nc.vector.memset(cz_b, 0.0)
            cz_acc.append(cz_b)

        red = []      # allreduced [ctxT | Z] per batch
        red_sb = []   # SBUF copies
        # qsm tiles outlive phase 1 (all transposes are deferred into the
        # AllReduce wait window), so pool at top level.
        qsm_pool = ctx.enter_context(tc.tile_pool(name="qsm", bufs=19))
        deferred_tr = {0: [], 1: []}   # (st, qsm) per batch

        def tile_transpose(b, st, qsm, pool):
            s0 = b * NL + st * 128
            trp = pool.tile([128, 4, 128], BF, tag="tr", name="trp")
            for c in range(4):
                nc.tensor.transpose(trp[:, c, :], qsm[:, 128 * c:128 * (c + 1)],
                                    ident)
            for c in range(4):
                nc.vector.tensor_copy(out=qsmT[:, c, s0:s0 + 128],
                                      in_=trp[:, c, :])

        # ---- phase 1: qkv + softmaxes + ctx/Z, tiles pipelined one deep ----
        with ExitStack() as p12:
            kvq_ps = p12.enter_context(tc.tile_pool(name="kvq_ps", bufs=2, space="PSUM"))
            cz_ps = p12.enter_context(tc.tile_pool(name="cz_ps", bufs=1, space="PSUM"))
            ek_pool = p12.enter_context(tc.tile_pool(name="ek", bufs=3))
            v_pool = p12.enter_context(tc.tile_pool(name="vp", bufs=3))
            eq_pool = p12.enter_context(tc.tile_pool(name="eq", bufs=3))
            qs_pool = p12.enter_context(tc.tile_pool(name="qs", bufs=3))

            def tile_mms(b, st):
                s0 = b * NL + st * 128
                kvq = kvq_ps.tile([128, 1536], F32, tag="kvq", name="kvq")
                for kk in range(8):
                    first, last = (kk == 0), (kk == 7)
                    nc.tensor.matmul(kvq[:, 0:512], lhsT=xt[:, kk, s0:s0 + 128],
                                     rhs=wq_sb[:, kk, 512:1024],
                                     start=first, stop=last)
                    nc.tensor.matmul(kvq[:, 512:1024], lhsT=xt[:, kk, s0:s0 + 128],
                                     rhs=wq_sb[:, kk, 1024:1536],
                                     start=first, stop=last)
                    nc.tensor.matmul(kvq[:, 1024:1536], lhsT=xt[:, kk, s0:s0 + 128],
                                     rhs=wq_sb[:, kk, 0:512],
                                     start=first, stop=last)
                return kvq

            def tile_elem(b, st, kvq):
                expk = ek_pool.tile([128, INNER], BF, tag="expk", name="expk")
                nc.scalar.activation(out=expk, in_=kvq[:, 0:512], func=EXP)
                vsb = v_pool.tile([128, INNER], BF, tag="v", name="vsb")
                nc.scalar.copy(out=vsb, in_=kvq[:, 512:1024])
                expq = eq_pool.tile([128, 8, 64], BF, tag="eq", name="expq")
                nc.scalar.activation(out=expq, in_=kvq[:, 1024:1536], func=EXP)
                qsum = qs_pool.tile([128, 8], F32, tag="qsum", name="qsum")
                nc.vector.tensor_reduce(qsum, expq, axis=AX_X, op=ADD)
                rq = qs_pool.tile([128, 8], F32, tag="rq", name="rq")
                nc.vector.reciprocal(rq, qsum)
                qsm = qsm_pool.tile([128, INNER], BF, tag="qsm", name="qsm")
                for h in range(H):
                    if h % 2 == 0:
                        nc.vector.tensor_scalar_mul(
                            qsm[:, 64 * h:64 * (h + 1)], expq[:, h, :],
                            rq[:, h:h + 1])
                    else:
                        nc.scalar.activation(
                            out=qsm[:, 64 * h:64 * (h + 1)], in_=expq[:, h, :],
                            func=COPY, scale=rq[:, h:h + 1])
                return expk, vsb, qsm

            def tile_deferred(b, st, expk, vsb, qsm):
                # ctx/Z matmuls for a finished tile; transposes inline for
                # batch 0, deferred into the AR-b1 window for batch 1
                cz = cz_ps.tile([128, 260], F32, tag="cz", name="cz")
                for h in range(H):
                    t, r = h // 2, h % 2
                    nc.tensor.matmul(
                        cz[64 * r:64 * (r + 1), 64 * t:64 * (t + 1)],
                        lhsT=vsb[:, 64 * h:64 * (h + 1)],
                        rhs=expk[:, 64 * h:64 * (h + 1)],
                        start=True, stop=True)
                for j in range(4):
                    nc.tensor.matmul(
                        cz[:, 256 + j:257 + j],
                        lhsT=expk[:, 128 * j:128 * (j + 1)], rhs=ones_bf,
                        start=True, stop=True)
                nc.vector.tensor_add(cz_acc[b], cz_acc[b], cz)
                deferred_tr[b].append((st, qsm))

            prev = None
            for b in range(B):
                for st in range(NT_B):
                    kvq = tile_mms(b, st)
                    if prev is not None:
                        tile_deferred(*prev)
                    ev = tile_elem(b, st, kvq)
                    prev = (b, st, *ev)
                tile_deferred(*prev)   # flush so the AR can be issued
                prev = None

                # keep collective-gated DMAs off the sync queue so output
                # DMAs emitted later never wait behind the AllReduce
                czbf = work.tile([128, 260], BF, tag=f"czbf{b}", name=f"czbf{b}")
                nc.vector.tensor_copy(out=czbf, in_=cz_acc[b])
                part_b = dram.tile([128, 260], BF, tag=f"part{b}", name=f"part{b}")
                red_b = dram.tile([128, 260], BF, tag=f"red{b}", name=f"red{b}")
                nc.gpsimd.dma_start(out=part_b, in_=czbf)
                nc.gpsimd.collective_compute(
                    "AllReduce", mybir.AluOpType.add,
                    replica_groups=[list(range(NCORES))],
                    ins=[part_b.opt()], outs=[red_b.opt()])
                red.append(red_b)
                red_c = work.tile([128, 260], BF, tag=f"red{b}", name=f"red_sb{b}")
                nc.gpsimd.dma_start(out=red_c, in_=red_b)
                red_sb.append(red_c)

        # ---- phase 2: M = (ctx @ W_out)/Z ; y = qsmT^T @ MZ ----
        with ExitStack() as pout:
            m_ps = pout.enter_context(tc.tile_pool(name="m_ps", bufs=2, space="PSUM"))
            y_ps = pout.enter_context(tc.tile_pool(name="y_ps", bufs=3, space="PSUM"))
            tr2_ps = pout.enter_context(tc.tile_pool(name="tr2_ps", bufs=2, space="PSUM"))
            ysb_pool = pout.enter_context(tc.tile_pool(name="ysb", bufs=4))

            def m_phase(b):
                rz = small.tile([128, 4], F32, tag="rz", name="rz")
                nc.vector.reciprocal(rz, red_sb[b][:, 256:260])
                m_sb = work.tile([128, 4, D], BF, tag="msb", name="m_sb")
                for t in range(4):
                    for cb in range(2):
                        mp = m_ps.tile([128, 512], F32, tag="mp", name="mp")
                        for r in range(2):
                            nc.tensor.matmul(
                                mp[64 * r:64 * (r + 1), :],
                                lhsT=red_sb[b][64 * r:64 * (r + 1), 64 * t:64 * (t + 1)],
                                rhs=wo_sb[64 * r:64 * (r + 1), t, cb * 512:(cb + 1) * 512],
                                start=True, stop=True)
                        if cb == 0:
                            nc.vector.tensor_scalar_mul(
                                m_sb[:, t, 0:512], mp, rz[:, t:t + 1])
                        else:
                            nc.scalar.activation(
                                out=m_sb[:, t, 512:1024], in_=mp,
                                func=COPY, scale=rz[:, t:t + 1])
                return m_sb

            def y_phase(b, m_sb):
                for mi in range(NT_B):
                    ysb = ysb_pool.tile([128, D], F32, tag="ysb", name="ysb")
                    for cb in range(2):
                        yp = y_ps.tile([128, 512], F32, tag="yp", name="yp")
                        for t in range(4):
                            nc.tensor.matmul(
                                yp, lhsT=qsmT[:, t, b * NL + mi * 128:
                                              b * NL + (mi + 1) * 128],
                                rhs=m_sb[:, t, cb * 512:(cb + 1) * 512],
                                start=(t == 0), stop=(t == 3))
                        if cb == 0:
                            nc.vector.tensor_copy(
                                out=ysb[:, 0:512], in_=yp)
                        else:
                            nc.scalar.copy(
                                out=ysb[:, 512:1024], in_=yp)
                        nc.sync.dma_start(
                            out=y[b * NL + mi * 128: b * NL + (mi + 1) * 128,
                                  cb * 512:(cb + 1) * 512],
                            in_=ysb[:, cb * 512:(cb + 1) * 512])

            # AR-b1 fill: M0 + all 16 tiles' transposes + y0 (~34us of PE)
            # cover the AR latency so M1/y1 start warm with no stall
            m0 = m_phase(0)
            for b in range(B):
                for st, qsm in deferred_tr[b]:
                    tile_transpose(b, st, qsm, tr2_ps)
            y_phase(0, m0)
            m1 = m_phase(1)
            y_phase(1, m1)


_COMPILED = None


def _build():
    global _COMPILED
    if _COMPILED is None:
        nc = bacc.Bacc("TRN2", target_bir_lowering=False, debug=False,
                       num_devices=NCORES)
        xT = nc.declare_dram_parameter("xT", [D, SEQ], BF, isOutput=False)
        wq = nc.declare_dram_parameter("wq", [D, 3 * INNER], BF, isOutput=False)
        wo = nc.declare_dram_parameter("wo", [INNER, D], BF, isOutput=False)
        y = nc.declare_dram_parameter("y", [SEQ, D], F32, isOutput=True)
        with tile.TileContext(nc) as tc:
            _body(tc, xT, wq, wo, y)
        nc.compile()
        _COMPILED = nc
    return _COMPILED


def _make_in_maps(x, W_qkv, W_out):
    wq_bf = np.ascontiguousarray(W_qkv).astype(bf16)
    wo_bf = np.ascontiguousarray(W_out).astype(bf16)
    in_maps = []
    for c in range(NCORES):
        rows = slice(c * NL, (c + 1) * NL)
        xs = np.concatenate([x[0, rows], x[1, rows]], axis=0)  # [2048, 1024]
        xT_bf = np.ascontiguousarray(xs.T).astype(bf16)        # [1024, 2048]
        in_maps.append({"xT": xT_bf, "wq": wq_bf, "wo": wo_bf})
    return in_maps


def _run(x, W_qkv, W_out, b_out, trace=False, **spmd_kwargs):
    nc = _build()
    in_maps = _make_in_maps(x, W_qkv, W_out)
    res = run_bass_kernel_spmd(nc, in_maps, list(range(NCORES)),
                               trace=trace, **spmd_kwargs)
    out = np.empty((B, N, D), np.float32)
    for c in range(NCORES):
        yc = res.results[c]["y"]
        rows = slice(c * NL, (c + 1) * NL)
        out[0, rows] = yc[:NL]
        out[1, rows] = yc[NL:]
    out += np.asarray(b_out, np.float32)[None, None, :]
    return out, res


def kernel(x, W_qkv, W_out, b_out):
    x = np.asarray(x, np.float32)
    out, _ = _run(x, np.asarray(W_qkv, np.float32),
                  np.asarray(W_out, np.float32),
                  np.asarray(b_out, np.float32))
    return out


# revision 44
# speedup vs baseline: 1.0622x; 1.0190x over previous
"""LinearAttention Trainium2 kernel (8 NeuronCores, sequence-sharded).

Reference computation (per batch b):
    qkv = x @ W_qkv; q,k,v split; per-head: softmax(q, dim=dh),
    softmax(k, dim=seq); ctx = k^T v; out = q_sm @ ctx; y = out @ W_out + b.

v4 dataflow per core (sequence shard of 1024 rows x 2 batches):
  phase 1 (per 128-row tile, software-pipelined one deep):
      qkv = xT.T @ W (24 N=512 matmuls per tile off one resident-xT
      LDWEIGHTS per k-chunk, f32 PSUM); exp_k / v-copy / exp_q on scalar;
      per-head q sums via one segmented DVE reduce + reciprocal; per-head
      scale split scalar/vector; the previous tile's ctxT/Z matmuls issue
      under the current tile's matmul chain so the PE never waits.
      A bf16 AllReduce of [ctxT | Z] is issued right after each batch's
      tiles (collective-gated DMAs ride the gpsimd queue so output DMAs
      never queue behind the collective).
  phase 2: M0 matmuls, then ALL 16 tiles' q_sm PE transposes (deferred
      from phase 1), then y0 — ~32us of PE work that covers the AR-b1
      latency so M1/y1 start warm with no stall. MZ_h = (ctx_h@W_out_h)/Z
      with 1/Z folded into the PSUM->SBUF copy scale; y = qsmT_t^T @ MZ_t
      summed over t; y copies alternate vector/scalar engines.
Host: shards/transposes/casts x, gathers per-core y shards, adds b_out.
"""
import numpy as np
import ml_dtypes
from contextlib import ExitStack

import concourse.bass as bass
import concourse.mybir as mybir
import concourse.tile as tile
from concourse import bacc
from concourse.bass_utils import run_bass_kernel_spmd
from concourse.masks import make_identity

bf16 = ml_dtypes.bfloat16
F32 = mybir.dt.float32
BF = mybir.dt.bfloat16
EXP = mybir.ActivationFunctionType.Exp
COPY = mybir.ActivationFunctionType.Copy
ADD = mybir.AluOpType.add
AX_X = mybir.AxisListType.X

B, N, D = 2, 8192, 1024
H, DH, INNER = 8, 64, 512
NCORES = 8
NL = N // NCORES            # 1024 seq rows per batch per core
SEQ = B * NL                # 2048 rows per core
NT_B = NL // 128            # 8 seq (128-row) tiles per batch


def _body(tc, xT, wq, wo, y):
    nc = tc.nc
    with ExitStack() as ctx:
        const = ctx.enter_context(tc.tile_pool(name="const", bufs=1))
        dram = ctx.enter_context(tc.tile_pool(name="dram", bufs=1, space="DRAM"))
        work = ctx.enter_context(tc.tile_pool(name="work", bufs=2))
        small = ctx.enter_context(tc.tile_pool(name="small", bufs=2))

        ident = const.tile([128, 128], BF)
        make_identity(nc, ident)
        ones_bf = const.tile([128, 1], BF)
        nc.vector.memset(ones_bf, 1.0)

        # interleave weight and xT loads so the first tiles start early
        xt = const.tile([128, 8, SEQ], BF)           # resident xT
        wq_sb = const.tile([128, 8, 3 * INNER], BF)
        wo_sb = const.tile([128, 4, D], BF)
        xT_r = xT[:].rearrange("(c p) s -> p c s", p=128)
        for kk in range(8):
            nc.sync.dma_start(out=wq_sb[:, kk, :],
                              in_=wq[128 * kk:128 * (kk + 1), :])
            nc.sync.dma_start(out=xt[:, kk, 0:256], in_=xT_r[:, kk, 0:256])
        for kk in range(8):
            nc.sync.dma_start(out=xt[:, kk, 256:NL], in_=xT_r[:, kk, 256:NL])
        for kk in range(8):
            nc.sync.dma_start(out=xt[:, kk, NL:SEQ], in_=xT_r[:, kk, NL:SEQ])
        for t in range(4):
            nc.sync.dma_start(out=wo_sb[:, t, :], in_=wo[128 * t:128 * (t + 1), :])

        qsmT = const.tile([128, 4, SEQ], BF)   # persistent q_sm^T
        cz_acc = []
        for b in range(B):
            cz_b = const.tile([128, 260], F32, tag=f"cz{b}", name=f"cz_acc{b}")
            nc.vector.memset(cz_b, 0.0)
            cz_acc.append(cz_b)

        red = []      # allreduced [ctxT | Z] per batch
        red_sb = []   # SBUF copies
        # qsm tiles outlive phase 1 (all transposes are deferred into the
        # AllReduce wait window), so pool at top level.
        qsm_pool = ctx.enter_context(tc.tile_pool(name="qsm", bufs=19))
        deferred_tr = {0: [], 1: []}   # (st, qsm) per batch

        def tile_transpose(b, st, qsm, pool):
            s0 = b * NL + st * 128
            trp = pool.tile([128, 4, 128], BF, tag="tr", name="trp")
            for c in range(4):
                nc.tensor.transpose(trp[:, c, :], qsm[:, 128 * c:128 * (c + 1)],
                                    ident)
            for c in range(4):
                nc.vector.tensor_copy(out=qsmT[:, c, s0:s0 + 128],
                                      in_=trp[:, c, :])

        # shared by phase 1 (batch 0) and the deferred batch-1 q pass
        eq_pool = ctx.enter_context(tc.tile_pool(name="eq", bufs=3))
        qs_pool = ctx.enter_context(tc.tile_pool(name="qs", bufs=3))

        def q_norm(qp_ap):
            # exp + per-head softmax normalization of one q tile
            expq = eq_pool.tile([128, 8, 64], BF, tag="eq", name="expq")
            nc.scalar.activation(out=expq, in_=qp_ap, func=EXP)
            qsum = qs_pool.tile([128, 8], F32, tag="qsum", name="qsum")
            nc.vector.tensor_reduce(qsum, expq, axis=AX_X, op=ADD)
            rq = qs_pool.tile([128, 8], F32, tag="rq", name="rq")
            nc.vector.reciprocal(rq, qsum)
            qsm = qsm_pool.tile([128, INNER], BF, tag="qsm", name="qsm")
            for h in range(H):
                if h % 2 == 0:
                    nc.vector.tensor_scalar_mul(
                        qsm[:, 64 * h:64 * (h + 1)], expq[:, h, :],
                        rq[:, h:h + 1])
                else:
                    nc.scalar.activation(
                        out=qsm[:, 64 * h:64 * (h + 1)], in_=expq[:, h, :],
                        func=COPY, scale=rq[:, h:h + 1])
            return qsm

        # ---- phase 1: qkv + softmaxes + ctx/Z, tiles pipelined one deep.
        # batch 1 computes only k/v (its q work is deferred into the
        # AllReduce-b1 wait window) so AR-b1 issues ~17us earlier. ----
        with ExitStack() as p12:
            kvq_ps = p12.enter_context(tc.tile_pool(name="kvq_ps", bufs=2, space="PSUM"))
            cz_ps = p12.enter_context(tc.tile_pool(name="cz_ps", bufs=1, space="PSUM"))
            ek_pool = p12.enter_context(tc.tile_pool(name="ek", bufs=3))
            v_pool = p12.enter_context(tc.tile_pool(name="vp", bufs=3))

            def tile_mms(b, st):
                s0 = b * NL + st * 128
                kvq = kvq_ps.tile([128, 1536], F32, tag="kvq", name="kvq")
                for kk in range(8):
                    first, last = (kk == 0), (kk == 7)
                    nc.tensor.matmul(kvq[:, 0:512], lhsT=xt[:, kk, s0:s0 + 128],
                                     rhs=wq_sb[:, kk, 512:1024],
                                     start=first, stop=last)
                    nc.tensor.matmul(kvq[:, 512:1024], lhsT=xt[:, kk, s0:s0 + 128],
                                     rhs=wq_sb[:, kk, 1024:1536],
                                     start=first, stop=last)
                    if b == 0:
                        nc.tensor.matmul(kvq[:, 1024:1536],
                                         lhsT=xt[:, kk, s0:s0 + 128],
                                         rhs=wq_sb[:, kk, 0:512],
                                         start=first, stop=last)
                return kvq

            def tile_elem(b, st, kvq):
                expk = ek_pool.tile([128, INNER], BF, tag="expk", name="expk")
                nc.scalar.activation(out=expk, in_=kvq[:, 0:512], func=EXP)
                vsb = v_pool.tile([128, INNER], BF, tag="v", name="vsb")
                nc.scalar.copy(out=vsb, in_=kvq[:, 512:1024])
                qsm = q_norm(kvq[:, 1024:1536]) if b == 0 else None
                return expk, vsb, qsm

            def tile_deferred(b, st, expk, vsb, qsm):
                # ctx/Z matmuls for a finished tile; transposes inline for
                # batch 0, deferred into the AR-b1 window for batch 1
                cz = cz_ps.tile([128, 260], F32, tag="cz", name="cz")
                for h in range(H):
                    t, r = h // 2, h % 2
                    nc.tensor.matmul(
                        cz[64 * r:64 * (r + 1), 64 * t:64 * (t + 1)],
                        lhsT=vsb[:, 64 * h:64 * (h + 1)],
                        rhs=expk[:, 64 * h:64 * (h + 1)],
                        start=True, stop=True)
                for j in range(4):
                    nc.tensor.matmul(
                        cz[:, 256 + j:257 + j],
                        lhsT=expk[:, 128 * j:128 * (j + 1)], rhs=ones_bf,
                        start=True, stop=True)
                nc.vector.tensor_add(cz_acc[b], cz_acc[b], cz)
                if qsm is not None:
                    deferred_tr[b].append((st, qsm))

            prev = None
            for b in range(B):
                for st in range(NT_B):
                    kvq = tile_mms(b, st)
                    if prev is not None:
                        tile_deferred(*prev)
                    ev = tile_elem(b, st, kvq)
                    prev = (b, st, *ev)
                tile_deferred(*prev)   # flush so the AR can be issued
                prev = None

                # keep collective-gated DMAs off the sync queue so output
                # DMAs emitted later never wait behind the AllReduce
                czbf = work.tile([128, 260], BF, tag=f"czbf{b}", name=f"czbf{b}")
                nc.vector.tensor_copy(out=czbf, in_=cz_acc[b])
                part_b = dram.tile([128, 260], BF, tag=f"part{b}", name=f"part{b}")
                red_b = dram.tile([128, 260], BF, tag=f"red{b}", name=f"red{b}")
                nc.gpsimd.dma_start(out=part_b, in_=czbf)
                nc.gpsimd.collective_compute(
                    "AllReduce", mybir.AluOpType.add,
                    replica_groups=[list(range(NCORES))],
                    ins=[part_b.opt()], outs=[red_b.opt()])
                red.append(red_b)
                red_c = work.tile([128, 260], BF, tag=f"red{b}", name=f"red_sb{b}")
                nc.gpsimd.dma_start(out=red_c, in_=red_b)
                red_sb.append(red_c)

        # ---- phase 2: deferred b1 q pass, M = (ctx @ W_out)/Z ;
        #      y = qsmT^T @ MZ ----
        with ExitStack() as pout:
            m_ps = pout.enter_context(tc.tile_pool(name="m_ps", bufs=2, space="PSUM"))
            y_ps = pout.enter_context(tc.tile_pool(name="y_ps", bufs=3, space="PSUM"))
            tr2_ps = pout.enter_context(tc.tile_pool(name="tr2_ps", bufs=1, space="PSUM"))
            q2_ps = pout.enter_context(tc.tile_pool(name="q2_ps", bufs=2, space="PSUM"))
            ysb_pool = pout.enter_context(tc.tile_pool(name="ysb", bufs=4))

            def q_pass_b1():
                # batch-1 q matmuls + softmax, pipelined one tile deep
                prevq = None
                for st in range(NT_B):
                    s0 = NL + st * 128
                    qp = q2_ps.tile([128, 512], F32, tag="qp", name="qp")
                    for kk in range(8):
                        nc.tensor.matmul(qp, lhsT=xt[:, kk, s0:s0 + 128],
                                         rhs=wq_sb[:, kk, 0:512],
                                         start=(kk == 0), stop=(kk == 7))
                    if prevq is not None:
                        deferred_tr[1].append((prevq[0], q_norm(prevq[1])))
                    prevq = (st, qp)
                deferred_tr[1].append((prevq[0], q_norm(prevq[1])))

            def m_phase(b):
                rz = small.tile([128, 4], F32, tag="rz", name="rz")
                nc.vector.reciprocal(rz, red_sb[b][:, 256:260])
                m_sb = work.tile([128, 4, D], BF, tag="msb", name="m_sb")
                for t in range(4):
                    for cb in range(2):
                        mp = m_ps.tile([128, 512], F32, tag="mp", name="mp")
                        for r in range(2):
                            nc.tensor.matmul(
                                mp[64 * r:64 * (r + 1), :],
                                lhsT=red_sb[b][64 * r:64 * (r + 1), 64 * t:64 * (t + 1)],
                                rhs=wo_sb[64 * r:64 * (r + 1), t, cb * 512:(cb + 1) * 512],
                                start=True, stop=True)
                        if cb == 0:
                            nc.vector.tensor_scalar_mul(
                                m_sb[:, t, 0:512], mp, rz[:, t:t + 1])
                        else:
                            nc.scalar.activation(
                                out=m_sb[:, t, 512:1024], in_=mp,
                                func=COPY, scale=rz[:, t:t + 1])
                return m_sb

            def y_phase(b, m_sb):
                for mi in range(NT_B):
                    ysb = ysb_pool.tile([128, D], F32, tag="ysb", name="ysb")
                    for cb in range(2):
                        yp = y_ps.tile([128, 512], F32, tag="yp", name="yp")
                        for t in range(4):
                            nc.tensor.matmul(
                                yp, lhsT=qsmT[:, t, b * NL + mi * 128:
                                              b * NL + (mi + 1) * 128],
                                rhs=m_sb[:, t, cb * 512:(cb + 1) * 512],
                                start=(t == 0), stop=(t == 3))
                        if cb == 0:
                            nc.vector.tensor_copy(
                                out=ysb[:, 0:512], in_=yp)
                        else:
                            nc.scalar.copy(
                                out=ysb[:, 512:1024], in_=yp)
                        nc.sync.dma_start(
                            out=y[b * NL + mi * 128: b * NL + (mi + 1) * 128,
                                  cb * 512:(cb + 1) * 512],
                            in_=ysb[:, cb * 512:(cb + 1) * 512])

            # AR-b1 fill: M0 + b1 q pass + all 16 tiles' transposes + y0
            # (~50us of PE) cover the AR latency even at its worst draw,
            # so M1/y1 start warm with no stall
            m0 = m_phase(0)
            q_pass_b1()
            for b in range(B):
                for st, qsm in deferred_tr[b]:
                    tile_transpose(b, st, qsm, tr2_ps)
            y_phase(0, m0)
            m1 = m_phase(1)
            y_phase(1, m1)


_COMPILED = None


def _build():
    global _COMPILED
    if _COMPILED is None:
        nc = bacc.Bacc("TRN2", target_bir_lowering=False, debug=False,
                       num_devices=NCORES)
        xT = nc.declare_dram_parameter("xT", [D, SEQ], BF, isOutput=False)
        wq = nc.declare_dram_parameter("wq", [D, 3 * INNER], BF, isOutput=False)
        wo = nc.declare_dram_parameter("wo", [INNER, D], BF, isOutput=False)
        y = nc.declare_dram_parameter("y", [SEQ, D], F32, isOutput=True)
        with tile.TileContext(nc) as tc:
            _body(tc, xT, wq, wo, y)
        nc.compile()
        _COMPILED = nc
    return _COMPILED


def _make_in_maps(x, W_qkv, W_out):
    wq_bf = np.ascontiguousarray(W_qkv).astype(bf16)
    wo_bf = np.ascontiguousarray(W_out).astype(bf16)
    in_maps = []
    for c in range(NCORES):
        rows = slice(c * NL, (c + 1) * NL)
        xs = np.concatenate([x[0, rows], x[1, rows]], axis=0)  # [2048, 1024]
        xT_bf = np.ascontiguousarray(xs.T).astype(bf16)        # [1024, 2048]
        in_maps.append({"xT": xT_bf, "wq": wq_bf, "wo": wo_bf})
    return in_maps


def _run(x, W_qkv, W_out, b_out, trace=False, **spmd_kwargs):
    nc = _build()
    in_maps = _make_in_maps(x, W_qkv, W_out)
    res = run_bass_kernel_spmd(nc, in_maps, list(range(NCORES)),
                               trace=trace, **spmd_kwargs)
    out = np.empty((B, N, D), np.float32)
    for c in range(NCORES):
        yc = res.results[c]["y"]
        rows = slice(c * NL, (c + 1) * NL)
        out[0, rows] = yc[:NL]
        out[1, rows] = yc[NL:]
    out += np.asarray(b_out, np.float32)[None, None, :]
    return out, res


def kernel(x, W_qkv, W_out, b_out):
    x = np.asarray(x, np.float32)
    out, _ = _run(x, np.asarray(W_qkv, np.float32),
                  np.asarray(W_out, np.float32),
                  np.asarray(b_out, np.float32))
    return out
